# revision 34
# baseline (speedup 1.0000x reference)
"""Trainium2 Bass kernel for nn_CaptionHead (segment_reduce).

Computes, for full-size inputs:
    point_feats = adapter_feats[v2p_map]            # [N_PTS, D]
    gathered    = point_feats[point_idx]            # [T, D]
    sums        = segment_sum(gathered, seg_ids, S) # [S, D]
    pooled      = l2norm(sums / max(counts, 1))     # == l2norm(sums)
    logits      = (pooled @ l2norm(ce).T) * exp(logit_scale)

Distribution: adapter_feats is sharded by voxel across the 8 cores
(25000 rows each, so shard-local indices fit the int16 dma_gather path).
Each point is routed (host-side index preprocessing) to the core owning
its voxel, grouped by 128-segment chunk, and padded so every
(core, chunk) cell has the same tile count -> one SPMD program.

On device, each core gathers its points' rows (bf16, 512B each -- the
max-rate descriptor size) with dma_gather, reduces them into per-chunk
[128, 256] PSUM accumulators with one-hot matmuls (the one-hot is built
by tensor_scalar is_equal against an iota row, with the per-point
segment id as the per-partition scalar operand -- keeps DVE in its 2x
perf mode), ReduceScatters the [S, D] partial sums so core r ends up
with segment rows [r*256, (r+1)*256), normalizes (the 1/count factor
cancels inside l2norm), and multiplies against the caption embeddings
with the row/col norm factors applied around the matmul.  Core r
returns logits rows [r*256, (r+1)*256); the host concatenates.
"""

import math

import numpy as np

N_VOX = 200000
N_PTS = 500000
T_FULL = 1000000
S_FULL = 2048
D_FULL = 256
N_CORES = 8
P = 128


def _preprocess(v2p_map, point_idx, seg_ids, n_cores, vox_per_core, n_chunks, trim=True):
    """Route points to voxel-owning cores, group by segment chunk, pad.

    Returns (idx16, segf, tiles_per_chunk, counts):
      idx16[m]: [128, NIDX//16] int16 shard-local voxel index per point in
                dma_gather's 16-partition-wrapped, 8x-replicated layout.
      segf[m]:  [128, NT] float32 chunk-local segment id per point
                (tile t, partition p -> point t*128 + p); -1 for padding.
    """
    v2p = np.asarray(v2p_map).astype(np.int64)
    pidx = np.asarray(point_idx).astype(np.int64)
    seg = np.asarray(seg_ids).astype(np.int64)
    cidx = v2p[pidx]                      # composed voxel index per point
    core = cidx // vox_per_core
    chunk = seg >> 7                      # 128 segments per chunk
    key = (core * n_chunks + chunk).astype(np.int32)
    order = np.argsort(key, kind="stable")
    cidx_s = cidx[order]
    seg_s = seg[order]
    counts = np.bincount(key, minlength=n_cores * n_chunks)
    tiles_per_chunk = max(1, math.ceil(counts.max() / P))
    npc = tiles_per_chunk * P
    lvox = np.full((n_cores, n_chunks, npc), -1 if trim else 0, np.int16)
    segl = np.full((n_cores, n_chunks, npc), -1.0, np.float32)
    offs = np.concatenate([[0], np.cumsum(counts)])
    for m in range(n_cores):
        for c in range(n_chunks):
            k = m * n_chunks + c
            a, b = offs[k], offs[k + 1]
            n = b - a
            lvox[m, c, :n] = (cidx_s[a:b] - m * vox_per_core).astype(np.int16)
            segl[m, c, :n] = (seg_s[a:b] - c * P).astype(np.float32)

    idx16 = []
    segf = []
    for m in range(n_cores):
        arr = lvox[m].reshape(-1, 16).T          # [16, NIDX//16]
        idx16.append(np.ascontiguousarray(np.tile(arr, (8, 1))))
        segf.append(np.ascontiguousarray(segl[m].reshape(-1, P).T))
    return idx16, segf, tiles_per_chunk, counts.reshape(n_cores, n_chunks)


def _build_nc(tiles_per_chunk, vox_per_core, D, S, n_cores, batch_tiles=8,
              main_reps=1, mode="full", single_core=False,
              gp_bufs=3, oh_bufs=3, acc_bufs=4):
    """mode: "full" | "nomm" (gathers only) | "nogather" (compute only)
    | "noonehot" (gather + matmul, constant weights).  main_reps repeats the
    main loop; with mode="full" the output stays correct (each rep recomputes
    the same sums; only the last is copied out)."""
    import concourse.bacc as bacc
    import concourse.mybir as mybir
    import concourse.tile as tile
    from concourse.masks import make_identity

    f32 = mybir.dt.float32
    bf16 = mybir.dt.bfloat16
    i16 = mybir.dt.int16
    i32 = mybir.dt.int32
    n_chunks = S // P
    NT = n_chunks * tiles_per_chunk            # total point tiles
    NIDX = NT * P                              # total gathered rows
    out_rows = S // n_cores                    # 256
    blk_tiles = out_rows // P                  # 2
    k_tiles = D // P                           # 2
    n_cols = 512                               # moving-operand tile width
    n_tiles_out = S // n_cols                  # 4

    nc = bacc.Bacc(
        "TRN2",
        target_bir_lowering=False,
        debug=False,
        enable_asserts=False,
        num_devices=n_cores,
        # SWDGE descriptor-ring carveout (bytes PER PARTITION): must hold
        # two in-flight dma_gathers of batch_tiles*128 descriptors each.
        dynamic_dma_scratch_size=16 * 2 * batch_tiles * P,
        # round-robin gathers over all 4 SWDGE queues: each queue's
        # descriptor generation runs on its own Q7 core pair.
        num_swdge_queues=4,
    )

    adapter = nc.dram_tensor("adapter", [vox_per_core, D], bf16, kind="ExternalInput")
    idx16 = nc.dram_tensor("idx16", [P, NIDX // 16], i16, kind="ExternalInput")
    segf = nc.dram_tensor("segf", [P, NT], f32, kind="ExternalInput")
    iota = nc.dram_tensor("iota", [P, P], bf16, kind="ExternalInput")
    cet = nc.dram_tensor("cet", [D, S], bf16, kind="ExternalInput")
    lsr = nc.dram_tensor("lsr", [P, 1], f32, kind="ExternalInput")
    n_batches = (tiles_per_chunk + batch_tiles - 1) // batch_tiles
    cnts = nc.dram_tensor("cnts", [1, n_chunks * n_batches], i32, kind="ExternalInput")
    out = nc.dram_tensor("logits_block", [out_rows, S], f32, kind="ExternalOutput")
    cc_in = nc.dram_tensor("cc_in", [S, D], f32, kind="Internal")
    half_rows = S // 2
    cc_out_h = [
        nc.dram_tensor(f"cc_out{h}", [half_rows // n_cores, D], f32, kind="Internal")
        for h in range(2)
    ]

    with tile.TileContext(nc) as tc:
        with (
            tc.tile_pool(name="const", bufs=1) as constp,
            tc.tile_pool(name="gather", bufs=gp_bufs) as gp,
            tc.tile_pool(name="oh", bufs=oh_bufs) as ohp,
            tc.tile_pool(name="misc", bufs=1) as miscp,
            tc.tile_pool(name="stage", bufs=2) as stp,
            tc.tile_pool(name="fin", bufs=1) as finp,
            tc.tile_pool(name="fpsum", bufs=1, space="PSUM") as fpp,
            tc.tile_pool(name="cestream", bufs=2) as cep,
        ):
            idx_sb = constp.tile([P, NIDX // 16], i16)
            nc.sync.dma_start(idx_sb[:], idx16.ap())
            segf_sb = constp.tile([P, NT], f32)
            nc.sync.dma_start(segf_sb[:], segf.ap())
            iota_sb = constp.tile([P, P], bf16)
            nc.sync.dma_start(iota_sb[:], iota.ap())
            cnt_sb = constp.tile([1, n_chunks * n_batches], i32)
            nc.sync.dma_start(cnt_sb[:], cnts.ap())
            ident = constp.tile([P, P], f32)
            make_identity(nc, ident[:])
            ident_bf = constp.tile([P, P], bf16)
            nc.vector.tensor_copy(out=ident_bf[:], in_=ident[:])

            sums_sb = None
            if mode == "nomm":
                sums_sb = miscp.tile([P, n_chunks * D], f32)

            # ---- prologue: everything independent of the gathered data ----
            # (emitted first so the in-order HWDGE sequencers don't stall
            # these loads behind the per-chunk staging DMAs)
            ls_sb = finp.tile([P, 1], f32)
            nc.sync.dma_start(ls_sb[:], lsr.ap())
            els = finp.tile([P, 1], f32)
            nc.scalar.activation(els[:], ls_sb[:], mybir.ActivationFunctionType.Exp)

            cet_sb = [finp.tile([P, S], bf16, tag=f"cet{k}", name=f"cet{k}")
                      for k in range(k_tiles)]
            for k in range(k_tiles):
                nc.sync.dma_start(cet_sb[k][:], cet.ap()[k * P : (k + 1) * P, :])

            # caption-embedding column scales 1/max(||ce_n||, 1e-12) computed
            # from cet alone: square on DVE (bf16 out), column-sum over the
            # d partition dim via a ones-vector matmul into a [1, S] row.
            sq_scr = finp.tile([P, D], f32)
            ones_col = finp.tile([P, 1], bf16)
            nc.vector.memset(ones_col[:], 1.0)
            csrow = finp.tile([1, S], f32)
            cet_sq = [cep.tile([P, S], bf16, tag="cesq", name=f"cesq{k}")
                      for k in range(k_tiles)]
            for k in range(k_tiles):
                nc.vector.tensor_tensor(
                    out=cet_sq[k][:],
                    in0=cet_sb[k][:],
                    in1=cet_sb[k][:],
                    op=mybir.AluOpType.mult,
                )
            for n in range(n_tiles_out):
                cs_ps = fpp.tile([1, n_cols], f32, tag="cs", name="cs", bufs=1)
                for k in range(k_tiles):
                    nc.tensor.matmul(
                        cs_ps[:],
                        lhsT=ones_col[:],
                        rhs=cet_sq[k][:, n * n_cols : (n + 1) * n_cols],
                        start=(k == 0),
                        stop=(k == k_tiles - 1),
                    )
                nc.vector.tensor_copy(
                    out=csrow[:, n * n_cols : (n + 1) * n_cols],
                    in_=cs_ps[:],
                )
            nc.vector.tensor_scalar_max(csrow[:], csrow[:], 1e-24)
            nc.scalar.sqrt(csrow[:], csrow[:])
            nc.vector.reciprocal(csrow[:], csrow[:])
            # broadcast col scales across partitions via K=1 matmul
            ones_row = finp.tile([1, P], f32)
            nc.vector.memset(ones_row[:], 1.0)
            colbc = finp.tile([P, S], f32)
            for n in range(n_tiles_out):
                cb_ps = fpp.tile([P, n_cols], f32, tag="ops", bufs=2)
                nc.tensor.matmul(
                    cb_ps[:],
                    lhsT=ones_row[:],
                    rhs=csrow[:, n * n_cols : (n + 1) * n_cols],
                    start=True,
                    stop=True,
                )
                nc.vector.tensor_copy(
                    out=colbc[:, n * n_cols : (n + 1) * n_cols], in_=cb_ps[:]
                )

            # ---- finale (emitted per half-block; m=0 mid-loop once RS0 is
            # long done, m=1 after the last ReduceScatter) ----
            pT = [finp.tile([P, out_rows], bf16, tag=f"pT{k}", name=f"pT{k}")
                  for k in range(k_tiles)]

            def emit_finale(m):
                blk = finp.tile([P, D], f32, tag=f"blk{m}", name=f"blk{m}")
                nc.sync.dma_start(blk[:], cc_out_h[m].ap())
                rs_inv = finp.tile([P, 1], f32, tag=f"ri{m}", name=f"ri{m}")
                nc.scalar.activation(
                    sq_scr[:],
                    blk[:],
                    mybir.ActivationFunctionType.Square,
                    accum_out=rs_inv[:],
                )
                nc.scalar.sqrt(rs_inv[:], rs_inv[:])
                nc.vector.tensor_scalar_max(rs_inv[:], rs_inv[:], 1e-12)
                nc.vector.reciprocal(rs_inv[:], rs_inv[:])
                nc.vector.tensor_tensor(
                    out=rs_inv[:], in0=rs_inv[:], in1=els[:],
                    op=mybir.AluOpType.mult,
                )
                blk_bf = finp.tile([P, D], bf16, tag=f"bb{m}", name=f"bb{m}")
                nc.vector.tensor_tensor(
                    out=blk_bf[:],
                    in0=blk[:],
                    in1=rs_inv[:].to_broadcast([P, D]),
                    op=mybir.AluOpType.mult,
                )
                for k in range(k_tiles):
                    t_ps = fpp.tile([P, P], bf16, tag="tps", name="tps",
                                    bufs=1)
                    nc.tensor.transpose(
                        t_ps[:], blk_bf[:, k * P : (k + 1) * P], ident_bf[:]
                    )
                    nc.vector.tensor_copy(
                        out=pT[k][:, m * P : (m + 1) * P], in_=t_ps[:]
                    )
                out_sb = finp.tile([P, S], f32, tag="os", name="os", bufs=1)
                for n in range(n_tiles_out):
                    o_ps = fpp.tile([P, n_cols], f32, tag="ops", name="ops",
                                    bufs=2)
                    for k in range(k_tiles):
                        nc.tensor.matmul(
                            o_ps[:],
                            lhsT=pT[k][:, m * P : (m + 1) * P],
                            rhs=cet_sb[k][:, n * n_cols : (n + 1) * n_cols],
                            start=(k == 0),
                            stop=(k == k_tiles - 1),
                        )
                    nc.vector.tensor_tensor(
                        out=out_sb[:, n * n_cols : (n + 1) * n_cols],
                        in0=o_ps[:],
                        in1=colbc[:, n * n_cols : (n + 1) * n_cols],
                        op=mybir.AluOpType.mult,
                    )
                nc.sync.dma_start(
                    out.ap()[m * P : (m + 1) * P, :], out_sb[:]
                )

            def emit_collective(h):
                lo = h * half_rows
                if single_core:
                    nc.sync.dma_start(
                        cc_out_h[h].ap(),
                        cc_in.ap()[lo : lo + P, :],
                    )
                else:
                    nc.gpsimd.collective_compute(
                        "ReduceScatter",
                        mybir.AluOpType.add,
                        replica_groups=[list(range(n_cores))],
                        ins=[cc_in.ap()[lo : lo + half_rows, :]],
                        outs=[cc_out_h[h].ap()],
                    )

            # ---- main: gather + one-hot matmul segment reduction ----
            for _slot in range(gp_bufs):
                g_init = gp.tile([P, batch_tiles, D], bf16, tag="g",
                                 name="g_init")
                nc.vector.memset(g_init[:], 0)
            g_static = None
            if mode == "nogather":
                g_static = miscp.tile([P, batch_tiles, D], bf16)
                nc.vector.memset(g_static[:], 1.0)
            if mode == "nomm":
                nc.vector.memset(sums_sb[:], 1.0)
            with tc.tile_pool(name="acc", bufs=acc_bufs, space="PSUM") as accp:
                for rep in range(main_reps):
                    for c in range(n_chunks):
                        last = rep == main_reps - 1
                        # deferred so the collective's sem wait (on the half-1
                        # staging DMAs) never holds the Pool SEQ while gathers
                        # are pending behind it
                        if last and c == n_chunks // 2 + 2:
                            emit_collective(0)
                        acc = None
                        if mode != "nomm":
                            acc = accp.tile([P, D], f32, tag="acc", name="acc")
                        done = 0
                        while done < tiles_per_chunk:
                            bt = min(batch_tiles, tiles_per_chunk - done)
                            if mode == "nogather":
                                g = g_static
                            else:
                                g = gp.tile([P, batch_tiles, D], bf16, tag="g",
                                            name="g")
                                col0 = (c * tiles_per_chunk + done) * P // 16
                                nidx = bt * P
                                bidx = c * n_batches + done // batch_tiles
                                vreg = nc.gpsimd.alloc_register()
                                nc.gpsimd.reg_load(
                                    vreg, cnt_sb[0:1, bidx : bidx + 1]
                                )
                                nc.gpsimd.dma_gather(
                                    out_ap=g[:, :bt, :],
                                    in_ap=adapter.ap(),
                                    idxs_ap=idx_sb[:, col0 : col0 + nidx // 16],
                                    num_idxs=nidx,
                                    num_idxs_reg=vreg,
                                    elem_size=D,
                                    queue_num=bidx % 4,
                                )
                                nc.gpsimd.free_register(vreg)
                            if mode == "nomm":
                                done += bt
                                continue
                            # one oh tile per batch: a single DVE->PE sem per
                            # batch_tiles matmuls (fine-grained waits cost
                            # ~100ns each on the consumer)
                            oh8 = None
                            if mode != "noonehot":
                                oh8 = ohp.tile([P, batch_tiles, P], bf16,
                                               tag="oh", name="oh")
                                for jj in range(bt):
                                    tglob = c * tiles_per_chunk + done + jj
                                    nc.vector.tensor_scalar(
                                        out=oh8[:, jj, :],
                                        in0=iota_sb[:],
                                        scalar1=segf_sb[:, tglob : tglob + 1],
                                        scalar2=None,
                                        op0=mybir.AluOpType.is_equal,
                                    )
                            for jj in range(bt):
                                oh = ident_bf if mode == "noonehot" else oh8[:, jj, :]
                                nc.tensor.matmul(
                                    acc[:],
                                    lhsT=oh,
                                    rhs=g[:, jj, :],
                                    start=(done + jj == 0),
                                    stop=(done + jj == tiles_per_chunk - 1),
                                )
                            done += bt
                        if rep == main_reps - 1:
                            # drain PSUM on the idle Activation engine, then
                            # stage via the ACT HWDGE queue so the SP queue
                            # stays free for other loads
                            if mode != "nomm":
                                st = stp.tile([P, D], f32, tag="st", name="st")
                                nc.scalar.activation(
                                    st[:], acc[:],
                                    mybir.ActivationFunctionType.Copy,
                                )
                                nc.scalar.dma_start(
                                    cc_in.ap()[c * P : (c + 1) * P, :],
                                    st[:],
                                )
                            else:
                                nc.scalar.dma_start(
                                    cc_in.ap()[c * P : (c + 1) * P, :],
                                    sums_sb[:, c * D : (c + 1) * D],
                                )
                # finale 0 depends only on RS0 (long done): run it during
                # RS1's ~18us, which also keeps PE's p-state warm for
                # finale 1.  The wait_until floor pins these to the end of
                # every engine queue -- without it the scheduler interleaves
                # the finale into the main loop right where RS0 completes,
                # and the in-order PE/Act queues then stall ~17us on it.
                # Emitted BEFORE the second collective so the DRAM dep
                # tracker cannot tie finale 0's load to RS1's completion.
                with tc.tile_wait_until(0.3 * main_reps):
                    emit_finale(0)
                emit_collective(1)
                with tc.tile_wait_until(0.35 * main_reps):
                    emit_finale(1)
    nc.compile()
    return nc


def _batch_counts(counts, tiles_per_chunk, batch_tiles, trim=True):
    """Per-(core, chunk, batch) valid index counts, clamped to the batch."""
    n_cores, n_chunks = counts.shape
    if not trim:
        counts = np.full_like(counts, tiles_per_chunk * P)
    n_batches = (tiles_per_chunk + batch_tiles - 1) // batch_tiles
    out = np.zeros((n_cores, n_chunks * n_batches), np.int32)
    for b in range(n_batches):
        start = b * batch_tiles * P
        width_tiles = min(batch_tiles, tiles_per_chunk - b * batch_tiles)
        cap = width_tiles * P
        vals = np.clip(counts - start, 0, cap)
        out[:, b::n_batches] = vals
    return out


def _make_in_maps(adapter_feats, caption_embed, logit_scale, idx16, segf,
                  n_cores, vox_per_core, counts=None, tiles_per_chunk=None,
                  batch_tiles=8, trim=True):
    import ml_dtypes

    bf = ml_dtypes.bfloat16
    af32 = np.asarray(adapter_feats, np.float32)
    af = np.ascontiguousarray(af32.astype(bf))               # [V, D] bf16
    cet_np = np.ascontiguousarray(np.asarray(caption_embed, np.float32).T.astype(bf))
    ls = np.asarray(logit_scale, np.float32).reshape(-1)[0]
    ls_rep = np.full((P, 1), ls, np.float32)
    iota_mat = np.ascontiguousarray(
        np.broadcast_to(np.arange(P, dtype=np.float32), (P, P)).astype(bf)
    )
    bc = _batch_counts(np.asarray(counts), tiles_per_chunk, batch_tiles, trim=trim)
    in_maps = []
    for m in range(n_cores):
        in_maps.append(
            {
                "adapter": af[m * vox_per_core : (m + 1) * vox_per_core],
                "idx16": idx16[m],
                "segf": segf[m],
                "iota": iota_mat,
                "cet": cet_np,
                "lsr": ls_rep,
                "cnts": bc[m : m + 1],
            }
        )
    return in_maps


def _run(inputs_dict, n_cores, vox_per_core, D, S, batch_tiles=8, trace=False):
    from concourse.bass_utils import run_bass_kernel_spmd

    trim = True
    idx16, segf, tiles_per_chunk, counts = _preprocess(
        inputs_dict["v2p_map"],
        inputs_dict["point_idx"],
        inputs_dict["seg_ids"],
        n_cores,
        vox_per_core,
        S // P,
        trim=True,
    )
    # a zero-valid-count gather would emit no descriptors and never fire its
    # completion semaphore; fall back to untrimmed padding in that case
    if _batch_counts(counts, tiles_per_chunk, batch_tiles, trim=True).min() == 0:
        trim = False
        idx16, segf, tiles_per_chunk, counts = _preprocess(
            inputs_dict["v2p_map"],
            inputs_dict["point_idx"],
            inputs_dict["seg_ids"],
            n_cores,
            vox_per_core,
            S // P,
            trim=False,
        )
    nc = _build_nc(tiles_per_chunk, vox_per_core, D, S, n_cores, batch_tiles)
    in_maps = _make_in_maps(
        inputs_dict["adapter_feats"],
        inputs_dict["caption_embed"],
        inputs_dict["logit_scale"],
        idx16,
        segf,
        n_cores,
        vox_per_core,
        counts=counts,
        tiles_per_chunk=tiles_per_chunk,
        batch_tiles=batch_tiles,
        trim=trim,
    )
    res = run_bass_kernel_spmd(
        nc, in_maps, core_ids=list(range(n_cores)), trace=trace
    )
    blocks = [res.results[m]["logits_block"] for m in range(n_cores)]
    return _assemble(blocks, S, n_cores), res


def _assemble(blocks, S, n_cores):
    """Core r's output block holds segment rows for chunk r (tile 0) and
    chunk n_cores+r (tile 1)."""
    half = S // 2
    full = np.empty((S, blocks[0].shape[1]), blocks[0].dtype)
    for r in range(n_cores):
        full[r * P : (r + 1) * P] = blocks[r][:P]
        full[half + r * P : half + (r + 1) * P] = blocks[r][P : 2 * P]
    return full


def kernel(adapter_feats, caption_embed, logit_scale, v2p_map, point_idx,
           seg_ids, num_segments=S_FULL, **_):
    logits, _res = _run(
        {
            "adapter_feats": adapter_feats,
            "caption_embed": caption_embed,
            "logit_scale": logit_scale,
            "v2p_map": v2p_map,
            "point_idx": point_idx,
            "seg_ids": seg_ids,
        },
        N_CORES,
        N_VOX // N_CORES,
        D_FULL,
        S_FULL,
    )
    return logits
